# revision 40
# baseline (speedup 1.0000x reference)
"""Trainium2 Bass kernel: causal multi-head self-attention (B=4, T=4096, D=128, H=4, dh=32).

Sharding: 8 cores = 4 batches x 2 head-pairs. Core c handles batch c//2, heads
{2*(c%2), 2*(c%2)+1}. Each core emits per-head unnormalized projections Y_h and
softmax denominators l_h; the host computes sum_h Y_h / l_h per batch.

S matmuls bf16 untiled full-array; O matmuls fp8e4m3 DoubleRow (one matmul
per PAIR of j-blocks: the two k-tiles hold the two blocks' V columns, halving
O-matmul columns vs bf16). Per (head, q-super of 512 queries), pairs descending:
  mask       : diag blocks get a prefix matmul accumulating a -30000 causal
               bias tile into S's masked/dead columns BEFORE the S matmul, so
               exp emits zeros there directly. (An affine_select on E breaks
               the DoubleRow consumers: gpsimd RMW writes stay invisible to
               PE/ScalarE readers for ~us despite correct semaphores.)
  S^T[j,q]   = ktz_jb(zero-padded K=128) @ qt -> PSUM pair tile [128,1024]
  E (fp8)    : head 0 + every 4th h1 pair on ScalarE (exp -> fp8e4); other h1
               pairs on VectorE via tensor_scalar round(s*a+b) -> int8 whose
               bits read as fp8e4m3 give 2^(s*log2e) (Schraudolph).
  O^T       += DoubleRow([vxA | vxB], [E_A | E_B]), vx block = [V8 | ones |
               Vres8 | 0] with Vres8 = fp8(V - fp8(V)): out rows 0:32 = Y8,
               row 32 = softmax denominator l, rows 33:65 = Y residual.
  proj       : wo rows 0:32 AND 33:65 = W_out^T (sums Y8 + Yres for free),
               ones at (32, 128) extracts l.
qkv runs one unit ahead (depth-2 xt prefetch) so early units never stall on
their own qkv evacuation chains.
"""

import math
import numpy as np

import concourse.bass as bass
import concourse.bacc as bacc
import concourse.mybir as mybir
import concourse.tile as tile
from concourse import bass_utils
import concourse.dve_ops as dve_ops
from concourse.dve_spec import Spec, Src0, C0, C1, relu, lower
from concourse.dve_uop import DveOpSpec

F32 = mybir.dt.float32
BF16 = mybir.dt.bfloat16
FP16 = mybir.dt.float16
FP8 = mybir.dt.float8e4
I16 = mybir.dt.int16
I8 = mybir.dt.int8
DRow = mybir.MatmulPerfMode.DoubleRow
Alu = mybir.AluOpType
Exp = mybir.ActivationFunctionType.Exp

B, T, D = 4, 4096, 128
H, DH = 4, 32
NCORES = 8
NQS = T // 512
SCALE = 1.0 / math.sqrt(DH)

# Schraudolph fp8e4m3-bit exp: int8_bits(e^s) ~= round(s*log2e*8 + (7+sigma)*8)
SIGMA = -0.03
SCHR_A = (1.0 / math.log(2.0)) * SCALE * 8.0
SCHR_B = (7.0 + SIGMA) * 8.0
MASKNEG = -30000.0


def _register_exp2():
    name = "EXP2_BITS_ANT"
    for op in dve_ops.OPS:
        if op.name == name:
            return op
    spec = Spec(body=relu(Src0 * C0 + C1))
    row = dve_ops._CUSTOM_DVE_ROW_BASE + len(dve_ops.OPS)
    assert row < 0x20
    shas = {}
    for ver in ("v3", "v4"):
        try:
            s = DveOpSpec(name=name, opcode=row, uops=lower(spec, ver=ver), rd1_en=False)
            shas[ver] = s.sha(ver)
        except Exception:
            pass
    dve_ops._SUB_OPCODE_FOR_NAME[name] = row
    op = dve_ops.DveOp(name, spec, subdim=False, uops_sha=shas)
    dve_ops.OPS.append(op)
    dve_ops.CUSTOM_DVE_SPECS[name] = spec
    return op


EXP2_OP = _register_exp2()


def build_program() -> bacc.Bacc:
    nc = bacc.Bacc("TRN2", target_bir_lowering=False, debug=False, num_devices=NCORES)

    xt_d = nc.dram_tensor("xt", [D, T], BF16, kind="ExternalInput").ap()
    wqk_d = [nc.dram_tensor(f"wqk{h}", [D, 64], BF16, kind="ExternalInput").ap() for h in range(2)]
    wv_d = nc.dram_tensor("wv", [D, 64], BF16, kind="ExternalInput").ap()
    wo_d = [nc.dram_tensor(f"wo{h}", [128, 132], FP16, kind="ExternalInput").ap() for h in range(2)]
    zpad_d = nc.dram_tensor("zpad", [96, T], BF16, kind="ExternalInput").ap()
    y_d = nc.dram_tensor("y", [2, NQS * 4, 128, 132], FP16, kind="ExternalOutput").ap()

    with tile.TileContext(nc) as tc:
        with (
            tc.tile_pool(name="const", bufs=1) as cpool,
            tc.tile_pool(name="epool", bufs=8) as epool,
            tc.tile_pool(name="ypool", bufs=4) as ypool,
            tc.tile_pool(name="psS", bufs=2, space="PSUM") as psS,
            tc.tile_pool(name="psO", bufs=4, space="PSUM") as psO,
        ):
            # ---- persistent SBUF ----
            xt = cpool.tile([D, T], BF16)
            wqkm = cpool.tile([D, 128], BF16)
            wv = cpool.tile([D, 64], BF16)
            wo = [cpool.tile([128, 132], FP16, name=f"wo{h}", tag=f"wo{h}") for h in range(2)]
            qt = [cpool.tile([128, T], BF16, name=f"qt{h}", tag=f"qt{h}") for h in range(2)]
            ktz = [cpool.tile([128, T], BF16, name=f"ktz{h}", tag=f"ktz{h}") for h in range(2)]
            vx = [cpool.tile([128, 128 * 32], FP8, name=f"vx{h}", tag=f"vx{h}") for h in range(2)]
            idn = cpool.tile([128, 128], BF16)
            rmask = cpool.tile([128, 1024], BF16)
            osb = [cpool.tile([128, T], FP16, name=f"osb{h}", tag=f"osb{h}") for h in range(2)]

            # ---- init loads ----
            nc.sync.dma_start(xt[:, 0:512], xt_d[:, 0:512])
            for h in range(2):
                nc.sync.dma_start(wqkm[:, 64 * h : 64 * h + 64], wqk_d[h][:, :])
            nc.sync.dma_start(xt[:, 512:1024], xt_d[:, 512:1024])
            for h in range(2):
                nc.scalar.dma_start(wo[h][:, :], wo_d[h][:, :])
            nc.scalar.dma_start(wv[:, :], wv_d[:, :])
            zq = [nc.sync, nc.gpsimd, nc.sync, nc.gpsimd]
            for h in range(2):
                # zero the padded contraction rows once; Q/K copies only write
                # rows 0:32. Issued from four different engine queues so the
                # descriptors dispatch in parallel instead of serializing the
                # startup on the Sync queue.
                zq[2 * h].dma_start(qt[h][32:128, :], zpad_d[:, :])
                zq[2 * h + 1].dma_start(ktz[h][32:128, :], zpad_d[:, :])
                # vx pattern: [V8_j | ones | Vres8 | zeros] per 128-col block.
                nc.gpsimd.memset(vx[h][:, 0:512], 0.0)
                for jb in range(4):
                    nc.gpsimd.memset(vx[h][:, 128 * jb + 32 : 128 * jb + 33], 1.0)
                nc.vector.memset(vx[h][:, 512:4096], 0.0)
                for jb in range(4, 32):
                    nc.gpsimd.memset(vx[h][:, 128 * jb + 32 : 128 * jb + 33], 1.0)
            # identity (for the mask prefix-matmul) and the causal bias tile:
            # rmask[p, u] = MASKNEG where u < p + 512 else 0. Block g of a
            # diag super reads rmask[:, 512-128g : 1024-128g] so that column
            # c of the block sees MASKNEG iff c < 128g + p (dead or above
            # the causal frontier). Built once at init (gpsimd writes have
            # ~30us to drain before first use).
            nc.vector.memset(idn[:, :], 1.0)
            nc.gpsimd.affine_select(idn[:, :], idn[:, :], pattern=[[1, 128]],
                                    compare_op=mybir.AluOpType.is_equal, fill=0.0,
                                    base=0, channel_multiplier=-1)
            nc.vector.memset(rmask[:, :], 0.0)
            nc.gpsimd.affine_select(rmask[:, :], rmask[:, :], pattern=[[1, 1024]],
                                    compare_op=mybir.AluOpType.is_ge, fill=MASKNEG,
                                    base=-512, channel_multiplier=-1)

            def copy_h(h, out, in_):
                """PSUM->SBUF evacuations: head 0 on ScalarE, head 1 on VectorE."""
                if h == 0:
                    nc.scalar.copy(out, in_)
                else:
                    nc.vector.tensor_copy(out, in_)

            def emit_qkv(qs):
                qsl = slice(512 * qs, 512 * (qs + 1))
                # both heads' Q/K in ONE matmul: wqkm cols = [q0|k0|q1|k1]
                p = psS.tile([128, 1024], F32, name="p", tag="s")
                nc.tensor.matmul(p[:, 0:512], wqkm[:, :], xt[:, qsl], start=True, stop=True)
                for h in range(2):
                    copy_h(h, qt[h][0:32, qsl], p[64 * h : 64 * h + 32, 0:512])
                    copy_h(h, ktz[h][0:32, qsl], p[64 * h + 32 : 64 * h + 64, 0:512])
                pv = psS.tile([128, 1024], F32, name="pv", tag="s")
                for k in range(4):
                    jsl = slice(512 * qs + 128 * k, 512 * qs + 128 * (k + 1))
                    nc.tensor.matmul(pv[:, 64 * k : 64 * k + 64], xt[:, jsl], wv[:, :], start=True, stop=True)
                for h in range(2):
                    srcv = pv[:, 0:256].rearrange("p (n s) -> p n s", s=64)[:, :, 32 * h : 32 * h + 32]
                    # block jb lands in slot jb^1 (pairwise swap, negative
                    # stride on the inner pair dim): vx pair slices then read
                    # [odd-block | even-block], matching the e k-tile order
                    seg = vx[h][:, 512 * qs : 512 * (qs + 1)]
                    dv8 = seg.rearrange("p (pr two s) -> p pr two s", two=2, s=128)[:, :, ::-1, 0:32]
                    dvr = seg.rearrange("p (pr two s) -> p pr two s", two=2, s=128)[:, :, ::-1, 33:65]
                    s4 = srcv.rearrange("p (pr two) s -> p pr two s", two=2)
                    nc.vector.tensor_copy(dv8, s4)
                    nc.vector.tensor_tensor(dvr, s4, dv8, Alu.subtract)
                # prefetch two supers ahead (qkv runs one unit early)
                if qs + 2 < NQS:
                    nsl = slice(512 * (qs + 2), 512 * (qs + 3))
                    nc.sync.dma_start(xt[:, nsl], xt_d[:, nsl])

            def emit_attn(qs, mid_cb=None):
                njb = 4 * (qs + 1)
                npairs = njb // 2
                o_ps = [psO.tile([128, 512], F32, name=f"o{h}", tag="o") for h in range(2)]
                s_tiles = {}
                e_tiles = {}
                blocks = list(range(njb - 1, -1, -1))  # descending: diag first
                pairs = [blocks[2 * i : 2 * i + 2] for i in range(npairs)]

                def emit_S(h, gi):
                    s = psS.tile([128, 1024], F32, name="s", tag="s")
                    s_tiles[(h, gi)] = s
                    for k, jb in enumerate(pairs[gi]):
                        g = jb - 4 * qs
                        kt = ktz[h][:, 128 * jb : 128 * (jb + 1)]
                        if g >= 0:
                            # causal+dead bias over [0, 128(g+1)) only: exp()
                            # emits zeros there directly (no affine_select on
                            # E -> no gpsimd write-visibility race with the
                            # fp8 DoubleRow O consumers). S accumulates onto
                            # it in the overlap, and covers the rest alone.
                            w = 128 * (g + 1)
                            lo = 128 * g
                            nc.tensor.matmul(
                                s[:, 512 * k : 512 * k + w],
                                idn[:, :],
                                rmask[:, 512 - lo : 512 - lo + w],
                                start=True,
                                stop=True,
                                skip_group_check=True,
                            )
                            nc.tensor.matmul(
                                s[:, 512 * k + lo : 512 * k + w],
                                kt,
                                qt[h][:, 512 * qs + lo : 512 * qs + w],
                                start=False,
                                stop=True,
                                skip_group_check=True,
                            )
                            if w < 512:
                                nc.tensor.matmul(
                                    s[:, 512 * k + w : 512 * (k + 1)],
                                    kt,
                                    qt[h][:, 512 * qs + w : 512 * (qs + 1)],
                                    start=True,
                                    stop=True,
                                )
                        else:
                            nc.tensor.matmul(
                                s[:, 512 * k : 512 * (k + 1)],
                                kt,
                                qt[h][:, 512 * qs : 512 * (qs + 1)],
                                start=True,
                                stop=True,
                            )

                def emit_E(h, gi):
                    s = s_tiles.pop((h, gi))
                    # one fp8 E tile per pair: [E_blk0 | E_blk1]. Masked/dead S
                    # columns hold MASKNEG so exp/schraudolph emit 0 / -0 bits;
                    # no post-mask pass needed (the affine_select's laggy gpsimd
                    # writes were what broke DoubleRow consumers).
                    e = epool.tile([128, 1024], FP8, name="e", tag="e")
                    e_tiles[(h, gi)] = e
                    if h == 0 or gi % 4 == 1:
                        nc.scalar.activation(e[:, :], s[:, :], Exp, scale=SCALE)
                    else:
                        nc.vector.tensor_scalar(e[:, :].bitcast(I8), s[:, :], SCHR_A, SCHR_B, Alu.mult, Alu.add)

                def emit_O(h, gi):
                    e = e_tiles.pop((h, gi))
                    # ONE fp8 DoubleRow matmul per pair covers BOTH j-blocks
                    # (k-tiles = the two blocks' vx columns); out rows
                    # 0:32=Y8, 32=l, 33:65=Y-residual from the fp8 V error.
                    jb0 = pairs[gi][1]  # lower/even block index
                    lhs = vx[h][:, 128 * jb0 : 128 * (jb0 + 2)].rearrange("p (two m) -> p two m", two=2)
                    nc.tensor.matmul(
                        o_ps[h][:, :],
                        lhs,
                        e[:, :].rearrange("p (two n) -> p two n", two=2),
                        start=(gi == 0),
                        stop=(gi == npairs - 1),
                        perf_mode=DRow,
                    )

                emit_S(0, 0)
                emit_S(1, 0)
                for gi in range(npairs):
                    emit_E(0, gi)
                    emit_E(1, gi)
                    if gi + 1 < npairs:
                        emit_S(0, gi + 1)
                        emit_S(1, gi + 1)
                    emit_O(0, gi)
                    # O(h1) deferred two periods: by then its Schraudolph E is
                    # long done, so the in-order PE queue never stalls on
                    # VectorE (epool's 6 slots = 3 per head exactly cover this)
                    if gi > 1:
                        emit_O(1, gi - 2)
                    if mid_cb is not None and gi == npairs // 2:
                        mid_cb()
                emit_O(1, npairs - 2)
                emit_O(1, npairs - 1)
                return o_ps

            def emit_osb(qs, o_ps):
                qsl = slice(512 * qs, 512 * (qs + 1))
                for h in range(2):
                    # both osb evacuations on VectorE: on ScalarE this copy sits
                    # just ahead of the next unit's first exp and blocks it
                    nc.vector.tensor_copy(osb[h][:, qsl], o_ps[h][:, :])

            def emit_proj(qs):
                for h in range(2):
                    p = psS.tile([128, 1024], F32, name="pp", tag="s")
                    for lqb in range(4):
                        qb = 4 * qs + lqb
                        nc.tensor.matmul(
                            p[:, 256 * lqb : 256 * lqb + 132],
                            osb[h][:, 128 * qb : 128 * (qb + 1)],
                            wo[h][:, :],
                            start=True,
                            stop=True,
                        )
                    yb = ypool.tile([128, 4, 132], FP16, name="yb", tag="y")
                    src = p[:, 0:1024].rearrange("p (n s) -> p n s", s=256)[:, :, 0:132]
                    copy_h(h, yb[:, :, :], src)
                    dst = y_d[h, 4 * qs : 4 * qs + 4].rearrange("n p c -> p n c")
                    nc.sync.dma_start(dst, yb[:, :, :])

            with nc.named_scope("attn"):
                o_prev = None
                for qs in range(NQS):
                    if qs == 0:
                        # depth-2 prefetch: qt/ktz/vx for qs+1 ready when unit
                        # qs starts, so early units never stall on qkv chains
                        emit_qkv(0)
                        emit_qkv(1)
                    if qs > 0:
                        emit_osb(qs - 1, o_prev)
                    # qkv for qs+2 emitted mid-unit so its PSUM->SBUF copies
                    # overlap this unit instead of stalling the boundary
                    cb = (lambda q=qs: emit_qkv(q + 2)) if qs + 2 < NQS else None
                    o_cur = emit_attn(qs, cb)
                    if qs > 0:
                        emit_proj(qs - 1)
                    o_prev = o_cur
                emit_osb(NQS - 1, o_prev)
                emit_proj(NQS - 1)

    nc.compile()
    return nc


def _to_bf16(x: np.ndarray) -> np.ndarray:
    import ml_dtypes

    return np.ascontiguousarray(x, dtype=np.float32).astype(ml_dtypes.bfloat16)


def make_in_maps(x: np.ndarray, W_qkv: np.ndarray, W_out: np.ndarray):
    x = np.asarray(x, dtype=np.float32)
    W_qkv = np.asarray(W_qkv, dtype=np.float32)
    W_out = np.asarray(W_out, dtype=np.float32)

    in_maps = []
    for c in range(NCORES):
        b = c // 2
        h0 = 2 * (c % 2)
        m = {"xt": _to_bf16(x[b].T), "zpad": _to_bf16(np.zeros((96, T), np.float32))}
        for i in range(2):
            h = h0 + i
            wqk = np.zeros((D, 64), np.float32)
            wqk[:, 0:32] = W_qkv[32 * h : 32 * h + 32, :].T
            wqk[:, 32:64] = W_qkv[128 + 32 * h : 128 + 32 * h + 32, :].T
            m[f"wqk{i}"] = _to_bf16(wqk)
            woi = np.zeros((128, 132), np.float32)
            # rows 0:32 multiply Y8 (fp8 V), rows 33:65 the fp8 V-residual
            woi[0:32, 0:128] = W_out[:, 32 * h : 32 * h + 32].T
            woi[33:65, 0:128] = W_out[:, 32 * h : 32 * h + 32].T
            woi[32, 128] = 1.0
            m[f"wo{i}"] = woi.astype(np.float16)
        m["wv"] = _to_bf16(W_qkv[256 + 32 * h0 : 256 + 32 * h0 + 64, :].T)
        in_maps.append(m)
    return in_maps


_PROGRAM_CACHE = {}


def kernel(x: np.ndarray, W_qkv: np.ndarray, W_out: np.ndarray, _trace=False, _tmpdir=None) -> np.ndarray:
    if "nc" not in _PROGRAM_CACHE:
        _PROGRAM_CACHE["nc"] = build_program()
    nc = _PROGRAM_CACHE["nc"]

    in_maps = make_in_maps(x, W_qkv, W_out)
    res = bass_utils.run_bass_kernel_spmd(
        nc, in_maps, core_ids=list(range(NCORES)), trace=_trace, tmpdir=_tmpdir
    )
    out = np.zeros((B, T, D), np.float32)
    for c in range(NCORES):
        b = c // 2
        y = np.asarray(res.results[c]["y"], dtype=np.float32)  # [2, 32, 128, 132]
        for i in range(2):
            yi = y[i].reshape(T, 132)
            out[b] += yi[:, 0:128] / yi[:, 128:129]
    if _trace:
        kernel.last_result = res
    return out



# revision 41
# speedup vs baseline: 1.2763x; 1.2763x over previous
"""Trainium2 Bass kernel: causal multi-head self-attention (B=4, T=4096, D=128, H=4, dh=32).

Sharding: 8 cores = 4 batches x 2 head-pairs. Core c handles batch c//2, heads
{2*(c%2), 2*(c%2)+1}. Each core emits per-head unnormalized projections Y_h and
softmax denominators l_h; the host computes sum_h Y_h / l_h per batch.

S matmuls bf16 untiled full-array; O matmuls fp8e4m3 DoubleRow (one matmul
per PAIR of j-blocks: the two k-tiles hold the two blocks' V columns, halving
O-matmul columns vs bf16). Per (head, q-super of 512 queries), pairs descending:
  mask       : diag blocks get a prefix matmul accumulating a -30000 causal
               bias tile into S's masked/dead columns BEFORE the S matmul, so
               exp emits zeros there directly. (An affine_select on E breaks
               the DoubleRow consumers: gpsimd RMW writes stay invisible to
               PE/ScalarE readers for ~us despite correct semaphores.)
  S^T[j,q]   = ktz_jb(zero-padded K=128) @ qt -> PSUM pair tile [128,1024]
  E (fp8)    : head 0 + every 4th h1 pair on ScalarE (exp -> fp8e4); other h1
               pairs on VectorE via tensor_scalar round(s*a+b) -> int8 whose
               bits read as fp8e4m3 give 2^(s*log2e) (Schraudolph).
  O^T       += DoubleRow([vxA | vxB], [E_A | E_B]), vx block = [V8 | ones |
               Vres8 | 0] with Vres8 = fp8(V - fp8(V)): out rows 0:32 = Y8,
               row 32 = softmax denominator l, rows 33:65 = Y residual.
  proj       : wo rows 0:32 AND 33:65 = W_out^T (sums Y8 + Yres for free),
               ones at (32, 128) extracts l.
qkv runs one unit ahead (depth-2 xt prefetch) so early units never stall on
their own qkv evacuation chains.
"""

import math
import numpy as np

import concourse.bass as bass
import concourse.bacc as bacc
import concourse.mybir as mybir
import concourse.tile as tile
from concourse import bass_utils
import concourse.dve_ops as dve_ops
from concourse.dve_spec import Spec, Src0, C0, C1, relu, lower
from concourse.dve_uop import DveOpSpec

F32 = mybir.dt.float32
BF16 = mybir.dt.bfloat16
FP16 = mybir.dt.float16
FP8 = mybir.dt.float8e4
I16 = mybir.dt.int16
I8 = mybir.dt.int8
DRow = mybir.MatmulPerfMode.DoubleRow
Alu = mybir.AluOpType
Exp = mybir.ActivationFunctionType.Exp

B, T, D = 4, 4096, 128
H, DH = 4, 32
NCORES = 8
NQS = T // 512
SCALE = 1.0 / math.sqrt(DH)

# Schraudolph fp8e4m3-bit exp: int8_bits(e^s) ~= round(s*log2e*8 + (7+sigma)*8)
SIGMA = -0.03
SCHR_A = (1.0 / math.log(2.0)) * SCALE * 8.0
SCHR_B = (7.0 + SIGMA) * 8.0
MASKNEG = -30000.0


def _register_exp2():
    name = "EXP2_BITS_ANT"
    for op in dve_ops.OPS:
        if op.name == name:
            return op
    spec = Spec(body=relu(Src0 * C0 + C1))
    row = dve_ops._CUSTOM_DVE_ROW_BASE + len(dve_ops.OPS)
    assert row < 0x20
    shas = {}
    for ver in ("v3", "v4"):
        try:
            s = DveOpSpec(name=name, opcode=row, uops=lower(spec, ver=ver), rd1_en=False)
            shas[ver] = s.sha(ver)
        except Exception:
            pass
    dve_ops._SUB_OPCODE_FOR_NAME[name] = row
    op = dve_ops.DveOp(name, spec, subdim=False, uops_sha=shas)
    dve_ops.OPS.append(op)
    dve_ops.CUSTOM_DVE_SPECS[name] = spec
    return op


EXP2_OP = _register_exp2()


def build_program() -> bacc.Bacc:
    nc = bacc.Bacc("TRN2", target_bir_lowering=False, debug=False, num_devices=NCORES)

    xt_d = nc.dram_tensor("xt", [D, T], BF16, kind="ExternalInput").ap()
    wqk_d = [nc.dram_tensor(f"wqk{h}", [D, 64], BF16, kind="ExternalInput").ap() for h in range(2)]
    wv_d = nc.dram_tensor("wv", [D, 64], BF16, kind="ExternalInput").ap()
    wo_d = [nc.dram_tensor(f"wo{h}", [128, 132], FP16, kind="ExternalInput").ap() for h in range(2)]
    zpad_d = nc.dram_tensor("zpad", [96, T], BF16, kind="ExternalInput").ap()
    y_d = nc.dram_tensor("y", [2, NQS * 4, 128, 132], FP16, kind="ExternalOutput").ap()

    with tile.TileContext(nc) as tc:
        with (
            tc.tile_pool(name="const", bufs=1) as cpool,
            tc.tile_pool(name="epool", bufs=8) as epool,
            tc.tile_pool(name="ypool", bufs=4) as ypool,
            tc.tile_pool(name="psS", bufs=3, space="PSUM") as psS,
            tc.tile_pool(name="psO", bufs=2, space="PSUM") as psO,
        ):
            # ---- persistent SBUF ----
            xt = cpool.tile([D, T], BF16)
            wqkm = cpool.tile([D, 128], BF16)
            wv = cpool.tile([D, 64], BF16)
            wo = [cpool.tile([128, 132], FP16, name=f"wo{h}", tag=f"wo{h}") for h in range(2)]
            qt = [cpool.tile([128, T], BF16, name=f"qt{h}", tag=f"qt{h}") for h in range(2)]
            ktz = [cpool.tile([128, T], BF16, name=f"ktz{h}", tag=f"ktz{h}") for h in range(2)]
            vx = [cpool.tile([128, 128 * 32], FP8, name=f"vx{h}", tag=f"vx{h}") for h in range(2)]
            idn = cpool.tile([128, 128], BF16)
            rmask = cpool.tile([128, 1024], BF16)
            osb = [cpool.tile([128, T], FP16, name=f"osb{h}", tag=f"osb{h}") for h in range(2)]

            # ---- init loads ----
            nc.sync.dma_start(xt[:, 0:512], xt_d[:, 0:512])
            for h in range(2):
                nc.sync.dma_start(wqkm[:, 64 * h : 64 * h + 64], wqk_d[h][:, :])
            nc.sync.dma_start(xt[:, 512:1024], xt_d[:, 512:1024])
            for h in range(2):
                nc.scalar.dma_start(wo[h][:, :], wo_d[h][:, :])
            nc.scalar.dma_start(wv[:, :], wv_d[:, :])
            zq = [nc.sync, nc.gpsimd, nc.sync, nc.gpsimd]
            for h in range(2):
                # zero the padded contraction rows once; Q/K copies only write
                # rows 0:32. Issued from four different engine queues so the
                # descriptors dispatch in parallel instead of serializing the
                # startup on the Sync queue.
                zq[2 * h].dma_start(qt[h][32:128, :], zpad_d[:, :])
                zq[2 * h + 1].dma_start(ktz[h][32:128, :], zpad_d[:, :])
                # vx pattern: [V8_j | ones | Vres8 | zeros] per 128-col block.
                nc.gpsimd.memset(vx[h][:, 0:512], 0.0)
                for jb in range(4):
                    nc.gpsimd.memset(vx[h][:, 128 * jb + 32 : 128 * jb + 33], 1.0)
                nc.vector.memset(vx[h][:, 512:4096], 0.0)
                for jb in range(4, 32):
                    nc.gpsimd.memset(vx[h][:, 128 * jb + 32 : 128 * jb + 33], 1.0)
            # identity (for the mask prefix-matmul) and the causal bias tile:
            # rmask[p, u] = MASKNEG where u < p + 512 else 0. Block g of a
            # diag super reads rmask[:, 512-128g : 1024-128g] so that column
            # c of the block sees MASKNEG iff c < 128g + p (dead or above
            # the causal frontier). Built once at init (gpsimd writes have
            # ~30us to drain before first use).
            nc.vector.memset(idn[:, :], 1.0)
            nc.gpsimd.affine_select(idn[:, :], idn[:, :], pattern=[[1, 128]],
                                    compare_op=mybir.AluOpType.is_equal, fill=0.0,
                                    base=0, channel_multiplier=-1)
            nc.vector.memset(rmask[:, :], 0.0)
            nc.gpsimd.affine_select(rmask[:, :], rmask[:, :], pattern=[[1, 1024]],
                                    compare_op=mybir.AluOpType.is_ge, fill=MASKNEG,
                                    base=-512, channel_multiplier=-1)

            def copy_h(h, out, in_):
                """PSUM->SBUF evacuations: head 0 on ScalarE, head 1 on VectorE."""
                if h == 0:
                    nc.scalar.copy(out, in_)
                else:
                    nc.vector.tensor_copy(out, in_)

            def emit_qkv(qs):
                qsl = slice(512 * qs, 512 * (qs + 1))
                # both heads' Q/K in ONE matmul: wqkm cols = [q0|k0|q1|k1]
                p = psS.tile([128, 1024], F32, name="p", tag="s")
                nc.tensor.matmul(p[:, 0:512], wqkm[:, :], xt[:, qsl], start=True, stop=True)
                for h in range(2):
                    copy_h(h, qt[h][0:32, qsl], p[64 * h : 64 * h + 32, 0:512])
                    copy_h(h, ktz[h][0:32, qsl], p[64 * h + 32 : 64 * h + 64, 0:512])
                pv = psS.tile([128, 1024], F32, name="pv", tag="s")
                for k in range(4):
                    jsl = slice(512 * qs + 128 * k, 512 * qs + 128 * (k + 1))
                    nc.tensor.matmul(pv[:, 64 * k : 64 * k + 64], xt[:, jsl], wv[:, :], start=True, stop=True)
                for h in range(2):
                    srcv = pv[:, 0:256].rearrange("p (n s) -> p n s", s=64)[:, :, 32 * h : 32 * h + 32]
                    # block jb lands in slot jb^1 (pairwise swap, negative
                    # stride on the inner pair dim): vx pair slices then read
                    # [odd-block | even-block], matching the e k-tile order
                    seg = vx[h][:, 512 * qs : 512 * (qs + 1)]
                    dv8 = seg.rearrange("p (pr two s) -> p pr two s", two=2, s=128)[:, :, ::-1, 0:32]
                    dvr = seg.rearrange("p (pr two s) -> p pr two s", two=2, s=128)[:, :, ::-1, 33:65]
                    s4 = srcv.rearrange("p (pr two) s -> p pr two s", two=2)
                    nc.vector.tensor_copy(dv8, s4)
                    nc.vector.tensor_tensor(dvr, s4, dv8, Alu.subtract)
                # prefetch two supers ahead (qkv runs one unit early)
                if qs + 2 < NQS:
                    nsl = slice(512 * (qs + 2), 512 * (qs + 3))
                    nc.sync.dma_start(xt[:, nsl], xt_d[:, nsl])

            def emit_attn(qs, mid_cb=None):
                njb = 4 * (qs + 1)
                npairs = njb // 2
                o_ps = [psO.tile([128, 512], F32, name=f"o{h}", tag="o") for h in range(2)]
                s_tiles = {}
                e_tiles = {}
                blocks = list(range(njb - 1, -1, -1))  # descending: diag first
                pairs = [blocks[2 * i : 2 * i + 2] for i in range(npairs)]

                def emit_S(h, gi):
                    s = psS.tile([128, 1024], F32, name="s", tag="s")
                    s_tiles[(h, gi)] = s
                    for k, jb in enumerate(pairs[gi]):
                        g = jb - 4 * qs
                        kt = ktz[h][:, 128 * jb : 128 * (jb + 1)]
                        if g >= 0:
                            # causal+dead bias over [0, 128(g+1)) only: exp()
                            # emits zeros there directly (no affine_select on
                            # E -> no gpsimd write-visibility race with the
                            # fp8 DoubleRow O consumers). S accumulates onto
                            # it in the overlap, and covers the rest alone.
                            w = 128 * (g + 1)
                            lo = 128 * g
                            nc.tensor.matmul(
                                s[:, 512 * k : 512 * k + w],
                                idn[:, :],
                                rmask[:, 512 - lo : 512 - lo + w],
                                start=True,
                                stop=True,
                                skip_group_check=True,
                            )
                            nc.tensor.matmul(
                                s[:, 512 * k + lo : 512 * k + w],
                                kt,
                                qt[h][:, 512 * qs + lo : 512 * qs + w],
                                start=False,
                                stop=True,
                                skip_group_check=True,
                            )
                            if w < 512:
                                nc.tensor.matmul(
                                    s[:, 512 * k + w : 512 * (k + 1)],
                                    kt,
                                    qt[h][:, 512 * qs + w : 512 * (qs + 1)],
                                    start=True,
                                    stop=True,
                                )
                        else:
                            nc.tensor.matmul(
                                s[:, 512 * k : 512 * (k + 1)],
                                kt,
                                qt[h][:, 512 * qs : 512 * (qs + 1)],
                                start=True,
                                stop=True,
                            )

                def emit_E(h, gi):
                    s = s_tiles.pop((h, gi))
                    # one fp8 E tile per pair: [E_blk0 | E_blk1]. Masked/dead S
                    # columns hold MASKNEG so exp/schraudolph emit 0 / -0 bits;
                    # no post-mask pass needed (the affine_select's laggy gpsimd
                    # writes were what broke DoubleRow consumers).
                    e = epool.tile([128, 1024], FP8, name="e", tag="e")
                    e_tiles[(h, gi)] = e
                    if h == 0 or gi % 4 == 1:
                        nc.scalar.activation(e[:, :], s[:, :], Exp, scale=SCALE)
                    else:
                        nc.vector.tensor_scalar(e[:, :].bitcast(I8), s[:, :], SCHR_A, SCHR_B, Alu.mult, Alu.add)

                def emit_O(h, gi):
                    e = e_tiles.pop((h, gi))
                    # ONE fp8 DoubleRow matmul per pair covers BOTH j-blocks
                    # (k-tiles = the two blocks' vx columns); out rows
                    # 0:32=Y8, 32=l, 33:65=Y-residual from the fp8 V error.
                    jb0 = pairs[gi][1]  # lower/even block index
                    lhs = vx[h][:, 128 * jb0 : 128 * (jb0 + 2)].rearrange("p (two m) -> p two m", two=2)
                    nc.tensor.matmul(
                        o_ps[h][:, :],
                        lhs,
                        e[:, :].rearrange("p (two n) -> p two n", two=2),
                        start=(gi == 0),
                        stop=(gi == npairs - 1),
                        perf_mode=DRow,
                    )

                emit_S(0, 0)
                emit_S(1, 0)
                for gi in range(npairs):
                    emit_E(0, gi)
                    emit_E(1, gi)
                    if gi + 1 < npairs:
                        emit_S(0, gi + 1)
                        emit_S(1, gi + 1)
                    emit_O(0, gi)
                    # O(h1) deferred two periods: by then its Schraudolph E is
                    # long done, so the in-order PE queue never stalls on
                    # VectorE (epool's 6 slots = 3 per head exactly cover this)
                    if gi > 1:
                        emit_O(1, gi - 2)
                    if mid_cb is not None and gi == npairs // 2:
                        mid_cb()
                emit_O(1, npairs - 2)
                emit_O(1, npairs - 1)
                return o_ps

            def emit_osb(qs, o_ps):
                qsl = slice(512 * qs, 512 * (qs + 1))
                for h in range(2):
                    # both osb evacuations on VectorE: on ScalarE this copy sits
                    # just ahead of the next unit's first exp and blocks it
                    nc.vector.tensor_copy(osb[h][:, qsl], o_ps[h][:, :])

            def emit_proj(qs):
                for h in range(2):
                    p = psS.tile([128, 1024], F32, name="pp", tag="s")
                    for lqb in range(4):
                        qb = 4 * qs + lqb
                        nc.tensor.matmul(
                            p[:, 256 * lqb : 256 * lqb + 132],
                            osb[h][:, 128 * qb : 128 * (qb + 1)],
                            wo[h][:, :],
                            start=True,
                            stop=True,
                        )
                    yb = ypool.tile([128, 4, 132], FP16, name="yb", tag="y")
                    src = p[:, 0:1024].rearrange("p (n s) -> p n s", s=256)[:, :, 0:132]
                    copy_h(h, yb[:, :, :], src)
                    dst = y_d[h, 4 * qs : 4 * qs + 4].rearrange("n p c -> p n c")
                    nc.sync.dma_start(dst, yb[:, :, :])

            with nc.named_scope("attn"):
                o_prev = None
                for qs in range(NQS):
                    if qs == 0:
                        # depth-2 prefetch: qt/ktz/vx for qs+1 ready when unit
                        # qs starts, so early units never stall on qkv chains
                        emit_qkv(0)
                        emit_qkv(1)
                    if qs > 0:
                        emit_osb(qs - 1, o_prev)
                    # qkv for qs+2 emitted mid-unit so its PSUM->SBUF copies
                    # overlap this unit instead of stalling the boundary
                    cb = (lambda q=qs: emit_qkv(q + 2)) if qs + 2 < NQS else None
                    o_cur = emit_attn(qs, cb)
                    if qs > 0:
                        emit_proj(qs - 1)
                    o_prev = o_cur
                emit_osb(NQS - 1, o_prev)
                emit_proj(NQS - 1)

    nc.compile()
    return nc


def _to_bf16(x: np.ndarray) -> np.ndarray:
    import ml_dtypes

    return np.ascontiguousarray(x, dtype=np.float32).astype(ml_dtypes.bfloat16)


def make_in_maps(x: np.ndarray, W_qkv: np.ndarray, W_out: np.ndarray):
    x = np.asarray(x, dtype=np.float32)
    W_qkv = np.asarray(W_qkv, dtype=np.float32)
    W_out = np.asarray(W_out, dtype=np.float32)

    in_maps = []
    for c in range(NCORES):
        b = c // 2
        h0 = 2 * (c % 2)
        m = {"xt": _to_bf16(x[b].T), "zpad": _to_bf16(np.zeros((96, T), np.float32))}
        for i in range(2):
            h = h0 + i
            wqk = np.zeros((D, 64), np.float32)
            wqk[:, 0:32] = W_qkv[32 * h : 32 * h + 32, :].T
            wqk[:, 32:64] = W_qkv[128 + 32 * h : 128 + 32 * h + 32, :].T
            m[f"wqk{i}"] = _to_bf16(wqk)
            woi = np.zeros((128, 132), np.float32)
            # rows 0:32 multiply Y8 (fp8 V), rows 33:65 the fp8 V-residual
            woi[0:32, 0:128] = W_out[:, 32 * h : 32 * h + 32].T
            woi[33:65, 0:128] = W_out[:, 32 * h : 32 * h + 32].T
            woi[32, 128] = 1.0
            m[f"wo{i}"] = woi.astype(np.float16)
        m["wv"] = _to_bf16(W_qkv[256 + 32 * h0 : 256 + 32 * h0 + 64, :].T)
        in_maps.append(m)
    return in_maps


_PROGRAM_CACHE = {}


def kernel(x: np.ndarray, W_qkv: np.ndarray, W_out: np.ndarray, _trace=False, _tmpdir=None) -> np.ndarray:
    if "nc" not in _PROGRAM_CACHE:
        _PROGRAM_CACHE["nc"] = build_program()
    nc = _PROGRAM_CACHE["nc"]

    in_maps = make_in_maps(x, W_qkv, W_out)
    res = bass_utils.run_bass_kernel_spmd(
        nc, in_maps, core_ids=list(range(NCORES)), trace=_trace, tmpdir=_tmpdir
    )
    out = np.zeros((B, T, D), np.float32)
    for c in range(NCORES):
        b = c // 2
        y = np.asarray(res.results[c]["y"], dtype=np.float32)  # [2, 32, 128, 132]
        for i in range(2):
            yi = y[i].reshape(T, 132)
            out[b] += yi[:, 0:128] / yi[:, 128:129]
    if _trace:
        kernel.last_result = res
    return out



# revision 42
# speedup vs baseline: 1.2811x; 1.0038x over previous
"""Trainium2 Bass kernel: causal multi-head self-attention (B=4, T=4096, D=128, H=4, dh=32).

Sharding: 8 cores = 4 batches x 2 head-pairs. Core c handles batch c//2, heads
{2*(c%2), 2*(c%2)+1}. Each core emits per-head unnormalized projections Y_h and
softmax denominators l_h; the host computes sum_h Y_h / l_h per batch.

S matmuls bf16 untiled full-array; O matmuls fp8e4m3 DoubleRow (one matmul
per PAIR of j-blocks: the two k-tiles hold the two blocks' V columns, halving
O-matmul columns vs bf16). Per (head, q-super of 512 queries), pairs descending:
  mask       : diag blocks get a prefix matmul accumulating a -30000 causal
               bias tile into S's masked/dead columns BEFORE the S matmul, so
               exp emits zeros there directly. (An affine_select on E breaks
               the DoubleRow consumers: gpsimd RMW writes stay invisible to
               PE/ScalarE readers for ~us despite correct semaphores.)
  S^T[j,q]   = ktz_jb(zero-padded K=128) @ qt -> PSUM pair tile [128,1024]
  E (fp8)    : head 0 + every 4th h1 pair on ScalarE (exp -> fp8e4); other h1
               pairs on VectorE via tensor_scalar round(s*a+b) -> int8 whose
               bits read as fp8e4m3 give 2^(s*log2e) (Schraudolph).
  O^T       += DoubleRow([vxA | vxB], [E_A | E_B]), vx block = [V8 | ones |
               Vres8 | 0] with Vres8 = fp8(V - fp8(V)): out rows 0:32 = Y8,
               row 32 = softmax denominator l, rows 33:65 = Y residual.
  proj       : wo rows 0:32 AND 33:65 = W_out^T (sums Y8 + Yres for free),
               ones at (32, 128) extracts l.
qkv runs one unit ahead (depth-2 xt prefetch) so early units never stall on
their own qkv evacuation chains.
"""

import math
import numpy as np

import concourse.bass as bass
import concourse.bacc as bacc
import concourse.mybir as mybir
import concourse.tile as tile
from concourse import bass_utils
import concourse.dve_ops as dve_ops
from concourse.dve_spec import Spec, Src0, C0, C1, relu, lower
from concourse.dve_uop import DveOpSpec

F32 = mybir.dt.float32
BF16 = mybir.dt.bfloat16
FP16 = mybir.dt.float16
FP8 = mybir.dt.float8e4
I16 = mybir.dt.int16
I8 = mybir.dt.int8
DRow = mybir.MatmulPerfMode.DoubleRow
Alu = mybir.AluOpType
Exp = mybir.ActivationFunctionType.Exp

B, T, D = 4, 4096, 128
H, DH = 4, 32
NCORES = 8
NQS = T // 512
SCALE = 1.0 / math.sqrt(DH)

# Schraudolph fp8e4m3-bit exp: int8_bits(e^s) ~= round(s*log2e*8 + (7+sigma)*8)
SIGMA = -0.03
SCHR_A = (1.0 / math.log(2.0)) * SCALE * 8.0
SCHR_B = (7.0 + SIGMA) * 8.0
MASKNEG = -30000.0


def _register_exp2():
    name = "EXP2_BITS_ANT"
    for op in dve_ops.OPS:
        if op.name == name:
            return op
    spec = Spec(body=relu(Src0 * C0 + C1))
    row = dve_ops._CUSTOM_DVE_ROW_BASE + len(dve_ops.OPS)
    assert row < 0x20
    shas = {}
    for ver in ("v3", "v4"):
        try:
            s = DveOpSpec(name=name, opcode=row, uops=lower(spec, ver=ver), rd1_en=False)
            shas[ver] = s.sha(ver)
        except Exception:
            pass
    dve_ops._SUB_OPCODE_FOR_NAME[name] = row
    op = dve_ops.DveOp(name, spec, subdim=False, uops_sha=shas)
    dve_ops.OPS.append(op)
    dve_ops.CUSTOM_DVE_SPECS[name] = spec
    return op


EXP2_OP = _register_exp2()


def build_program() -> bacc.Bacc:
    nc = bacc.Bacc("TRN2", target_bir_lowering=False, debug=False, num_devices=NCORES)

    xt_d = nc.dram_tensor("xt", [D, T], BF16, kind="ExternalInput").ap()
    wqk_d = [nc.dram_tensor(f"wqk{h}", [D, 64], BF16, kind="ExternalInput").ap() for h in range(2)]
    wv_d = nc.dram_tensor("wv", [D, 64], BF16, kind="ExternalInput").ap()
    wo_d = [nc.dram_tensor(f"wo{h}", [128, 132], FP16, kind="ExternalInput").ap() for h in range(2)]
    zpad_d = nc.dram_tensor("zpad", [96, T], BF16, kind="ExternalInput").ap()
    y_d = nc.dram_tensor("y", [2, NQS * 4, 128, 132], FP16, kind="ExternalOutput").ap()

    with tile.TileContext(nc) as tc:
        with (
            tc.tile_pool(name="const", bufs=1) as cpool,
            tc.tile_pool(name="epool", bufs=8) as epool,
            tc.tile_pool(name="ypool", bufs=4) as ypool,
            tc.tile_pool(name="psS", bufs=3, space="PSUM") as psS,
            tc.tile_pool(name="psO", bufs=2, space="PSUM") as psO,
        ):
            # ---- persistent SBUF ----
            xt = cpool.tile([D, T], BF16)
            wqkm = cpool.tile([D, 128], BF16)
            wv = cpool.tile([D, 64], BF16)
            wo = [cpool.tile([128, 132], FP16, name=f"wo{h}", tag=f"wo{h}") for h in range(2)]
            qt = [cpool.tile([128, T], BF16, name=f"qt{h}", tag=f"qt{h}") for h in range(2)]
            ktz = [cpool.tile([128, T], BF16, name=f"ktz{h}", tag=f"ktz{h}") for h in range(2)]
            vx = [cpool.tile([128, 128 * 32], FP8, name=f"vx{h}", tag=f"vx{h}") for h in range(2)]
            idn = cpool.tile([128, 128], BF16)
            rmask = cpool.tile([128, 1024], BF16)
            osb = [cpool.tile([128, T], FP16, name=f"osb{h}", tag=f"osb{h}") for h in range(2)]

            # ---- init loads ----
            nc.sync.dma_start(xt[:, 0:512], xt_d[:, 0:512])
            for h in range(2):
                nc.sync.dma_start(wqkm[:, 64 * h : 64 * h + 64], wqk_d[h][:, :])
            nc.sync.dma_start(xt[:, 512:1024], xt_d[:, 512:1024])
            for h in range(2):
                nc.scalar.dma_start(wo[h][:, :], wo_d[h][:, :])
            nc.scalar.dma_start(wv[:, :], wv_d[:, :])
            zq = [nc.sync, nc.gpsimd, nc.sync, nc.gpsimd]
            for h in range(2):
                # zero the padded contraction rows once; Q/K copies only write
                # rows 0:32. Issued from four different engine queues so the
                # descriptors dispatch in parallel instead of serializing the
                # startup on the Sync queue.
                zq[2 * h].dma_start(qt[h][32:128, :], zpad_d[:, :])
                zq[2 * h + 1].dma_start(ktz[h][32:128, :], zpad_d[:, :])
                # vx pattern: [V8_j | ones | Vres8 | zeros] per 128-col block.
                nc.gpsimd.memset(vx[h][:, 0:512], 0.0)
                for jb in range(4):
                    nc.gpsimd.memset(vx[h][:, 128 * jb + 32 : 128 * jb + 33], 1.0)
                nc.vector.memset(vx[h][:, 512:4096], 0.0)
                for jb in range(4, 32):
                    nc.gpsimd.memset(vx[h][:, 128 * jb + 32 : 128 * jb + 33], 1.0)
            # identity (for the mask prefix-matmul) and the causal bias tile:
            # rmask[p, u] = MASKNEG where u < p + 512 else 0. Block g of a
            # diag super reads rmask[:, 512-128g : 1024-128g] so that column
            # c of the block sees MASKNEG iff c < 128g + p (dead or above
            # the causal frontier). Built once at init (gpsimd writes have
            # ~30us to drain before first use).
            nc.vector.memset(idn[:, :], 1.0)
            nc.gpsimd.affine_select(idn[:, :], idn[:, :], pattern=[[1, 128]],
                                    compare_op=mybir.AluOpType.is_equal, fill=0.0,
                                    base=0, channel_multiplier=-1)
            nc.vector.memset(rmask[:, :], 0.0)
            nc.gpsimd.affine_select(rmask[:, :], rmask[:, :], pattern=[[1, 1024]],
                                    compare_op=mybir.AluOpType.is_ge, fill=MASKNEG,
                                    base=-512, channel_multiplier=-1)

            def copy_h(h, out, in_):
                """PSUM->SBUF evacuations: head 0 on ScalarE, head 1 on VectorE."""
                if h == 0:
                    nc.scalar.copy(out, in_)
                else:
                    nc.vector.tensor_copy(out, in_)

            def emit_qkv(qs):
                qsl = slice(512 * qs, 512 * (qs + 1))
                # both heads' Q/K in ONE matmul: wqkm cols = [q0|k0|q1|k1]
                p = psS.tile([128, 1024], F32, name="p", tag="s")
                nc.tensor.matmul(p[:, 0:512], wqkm[:, :], xt[:, qsl], start=True, stop=True)
                for h in range(2):
                    copy_h(h, qt[h][0:32, qsl], p[64 * h : 64 * h + 32, 0:512])
                    copy_h(h, ktz[h][0:32, qsl], p[64 * h + 32 : 64 * h + 64, 0:512])
                pv = psS.tile([128, 1024], F32, name="pv", tag="s")
                for k in range(4):
                    jsl = slice(512 * qs + 128 * k, 512 * qs + 128 * (k + 1))
                    nc.tensor.matmul(pv[:, 64 * k : 64 * k + 64], xt[:, jsl], wv[:, :], start=True, stop=True)
                for h in range(2):
                    srcv = pv[:, 0:256].rearrange("p (n s) -> p n s", s=64)[:, :, 32 * h : 32 * h + 32]
                    # block jb lands in slot jb^1 (pairwise swap, negative
                    # stride on the inner pair dim): vx pair slices then read
                    # [odd-block | even-block], matching the e k-tile order
                    seg = vx[h][:, 512 * qs : 512 * (qs + 1)]
                    dv8 = seg.rearrange("p (pr two s) -> p pr two s", two=2, s=128)[:, :, ::-1, 0:32]
                    dvr = seg.rearrange("p (pr two s) -> p pr two s", two=2, s=128)[:, :, ::-1, 33:65]
                    s4 = srcv.rearrange("p (pr two) s -> p pr two s", two=2)
                    nc.vector.tensor_copy(dv8, s4)
                    nc.vector.tensor_tensor(dvr, s4, dv8, Alu.subtract)
                # prefetch two supers ahead (qkv runs one unit early)
                if qs + 2 < NQS:
                    nsl = slice(512 * (qs + 2), 512 * (qs + 3))
                    nc.sync.dma_start(xt[:, nsl], xt_d[:, nsl])

            def emit_attn(qs, mid_cb=None):
                njb = 4 * (qs + 1)
                npairs = njb // 2
                o_ps = [psO.tile([128, 512], F32, name=f"o{h}", tag="o") for h in range(2)]
                s_tiles = {}
                e_tiles = {}
                blocks = list(range(njb - 1, -1, -1))  # descending: diag first
                pairs = [blocks[2 * i : 2 * i + 2] for i in range(npairs)]

                def emit_S(h, gi):
                    s = psS.tile([128, 1024], F32, name="s", tag="s")
                    s_tiles[(h, gi)] = s
                    for k, jb in enumerate(pairs[gi]):
                        g = jb - 4 * qs
                        kt = ktz[h][:, 128 * jb : 128 * (jb + 1)]
                        if g >= 0:
                            # causal+dead bias over [0, 128(g+1)) only: exp()
                            # emits zeros there directly (no affine_select on
                            # E -> no gpsimd write-visibility race with the
                            # fp8 DoubleRow O consumers). S accumulates onto
                            # it in the overlap, and covers the rest alone.
                            w = 128 * (g + 1)
                            lo = 128 * g
                            nc.tensor.matmul(
                                s[:, 512 * k : 512 * k + w],
                                idn[:, :],
                                rmask[:, 512 - lo : 512 - lo + w],
                                start=True,
                                stop=True,
                                skip_group_check=True,
                            )
                            nc.tensor.matmul(
                                s[:, 512 * k + lo : 512 * k + w],
                                kt,
                                qt[h][:, 512 * qs + lo : 512 * qs + w],
                                start=False,
                                stop=True,
                                skip_group_check=True,
                            )
                            if w < 512:
                                nc.tensor.matmul(
                                    s[:, 512 * k + w : 512 * (k + 1)],
                                    kt,
                                    qt[h][:, 512 * qs + w : 512 * (qs + 1)],
                                    start=True,
                                    stop=True,
                                )
                        else:
                            nc.tensor.matmul(
                                s[:, 512 * k : 512 * (k + 1)],
                                kt,
                                qt[h][:, 512 * qs : 512 * (qs + 1)],
                                start=True,
                                stop=True,
                            )

                def emit_E(h, gi):
                    s = s_tiles.pop((h, gi))
                    # one fp8 E tile per pair: [E_blk0 | E_blk1]. Masked/dead S
                    # columns hold MASKNEG so exp/schraudolph emit 0 / -0 bits;
                    # no post-mask pass needed (the affine_select's laggy gpsimd
                    # writes were what broke DoubleRow consumers).
                    e = epool.tile([128, 1024], FP8, name="e", tag="e")
                    e_tiles[(h, gi)] = e
                    if h == 0 or gi % 4 == 1:
                        nc.scalar.activation(e[:, :], s[:, :], Exp, scale=SCALE)
                    else:
                        nc.vector.tensor_scalar(e[:, :].bitcast(I8), s[:, :], SCHR_A, SCHR_B, Alu.mult, Alu.add)

                def emit_O(h, gi):
                    e = e_tiles.pop((h, gi))
                    # ONE fp8 DoubleRow matmul per pair covers BOTH j-blocks
                    # (k-tiles = the two blocks' vx columns); out rows
                    # 0:32=Y8, 32=l, 33:65=Y-residual from the fp8 V error.
                    jb0 = pairs[gi][1]  # lower/even block index
                    lhs = vx[h][:, 128 * jb0 : 128 * (jb0 + 2)].rearrange("p (two m) -> p two m", two=2)
                    nc.tensor.matmul(
                        o_ps[h][:, :],
                        lhs,
                        e[:, :].rearrange("p (two n) -> p two n", two=2),
                        start=(gi == 0),
                        stop=(gi == npairs - 1),
                        perf_mode=DRow,
                    )

                emit_S(0, 0)
                emit_S(1, 0)
                for gi in range(npairs):
                    emit_E(0, gi)
                    emit_E(1, gi)
                    if gi + 1 < npairs:
                        emit_S(0, gi + 1)
                        emit_S(1, gi + 1)
                    emit_O(0, gi)
                    # O(h1) deferred three periods: by then its Schraudolph E
                    # is long done, so the in-order PE queue never stalls on
                    # VectorE (the DR O-matmuls are half as long, so the PE
                    # arrives sooner; epool's 8 slots cover the deeper window)
                    if gi > 2:
                        emit_O(1, gi - 3)
                    if mid_cb is not None and gi == npairs // 2:
                        mid_cb()
                if npairs >= 3:
                    emit_O(1, npairs - 3)
                emit_O(1, npairs - 2)
                emit_O(1, npairs - 1)
                return o_ps

            def emit_osb(qs, o_ps):
                qsl = slice(512 * qs, 512 * (qs + 1))
                for h in range(2):
                    # both osb evacuations on VectorE: on ScalarE this copy sits
                    # just ahead of the next unit's first exp and blocks it
                    nc.vector.tensor_copy(osb[h][:, qsl], o_ps[h][:, :])

            def emit_proj(qs):
                for h in range(2):
                    p = psS.tile([128, 1024], F32, name="pp", tag="s")
                    for lqb in range(4):
                        qb = 4 * qs + lqb
                        nc.tensor.matmul(
                            p[:, 256 * lqb : 256 * lqb + 132],
                            osb[h][:, 128 * qb : 128 * (qb + 1)],
                            wo[h][:, :],
                            start=True,
                            stop=True,
                        )
                    yb = ypool.tile([128, 4, 132], FP16, name="yb", tag="y")
                    src = p[:, 0:1024].rearrange("p (n s) -> p n s", s=256)[:, :, 0:132]
                    copy_h(h, yb[:, :, :], src)
                    dst = y_d[h, 4 * qs : 4 * qs + 4].rearrange("n p c -> p n c")
                    nc.sync.dma_start(dst, yb[:, :, :])

            with nc.named_scope("attn"):
                o_prev = None
                for qs in range(NQS):
                    if qs == 0:
                        # depth-2 prefetch: qt/ktz/vx for qs+1 ready when unit
                        # qs starts, so early units never stall on qkv chains
                        emit_qkv(0)
                        emit_qkv(1)
                    if qs > 0:
                        emit_osb(qs - 1, o_prev)
                    # qkv for qs+2 emitted mid-unit so its PSUM->SBUF copies
                    # overlap this unit instead of stalling the boundary
                    cb = (lambda q=qs: emit_qkv(q + 2)) if qs + 2 < NQS else None
                    o_cur = emit_attn(qs, cb)
                    if qs > 0:
                        emit_proj(qs - 1)
                    o_prev = o_cur
                emit_osb(NQS - 1, o_prev)
                emit_proj(NQS - 1)

    nc.compile()
    return nc


def _to_bf16(x: np.ndarray) -> np.ndarray:
    import ml_dtypes

    return np.ascontiguousarray(x, dtype=np.float32).astype(ml_dtypes.bfloat16)


def make_in_maps(x: np.ndarray, W_qkv: np.ndarray, W_out: np.ndarray):
    x = np.asarray(x, dtype=np.float32)
    W_qkv = np.asarray(W_qkv, dtype=np.float32)
    W_out = np.asarray(W_out, dtype=np.float32)

    in_maps = []
    for c in range(NCORES):
        b = c // 2
        h0 = 2 * (c % 2)
        m = {"xt": _to_bf16(x[b].T), "zpad": _to_bf16(np.zeros((96, T), np.float32))}
        for i in range(2):
            h = h0 + i
            wqk = np.zeros((D, 64), np.float32)
            wqk[:, 0:32] = W_qkv[32 * h : 32 * h + 32, :].T
            wqk[:, 32:64] = W_qkv[128 + 32 * h : 128 + 32 * h + 32, :].T
            m[f"wqk{i}"] = _to_bf16(wqk)
            woi = np.zeros((128, 132), np.float32)
            # rows 0:32 multiply Y8 (fp8 V), rows 33:65 the fp8 V-residual
            woi[0:32, 0:128] = W_out[:, 32 * h : 32 * h + 32].T
            woi[33:65, 0:128] = W_out[:, 32 * h : 32 * h + 32].T
            woi[32, 128] = 1.0
            m[f"wo{i}"] = woi.astype(np.float16)
        m["wv"] = _to_bf16(W_qkv[256 + 32 * h0 : 256 + 32 * h0 + 64, :].T)
        in_maps.append(m)
    return in_maps


_PROGRAM_CACHE = {}


def kernel(x: np.ndarray, W_qkv: np.ndarray, W_out: np.ndarray, _trace=False, _tmpdir=None) -> np.ndarray:
    if "nc" not in _PROGRAM_CACHE:
        _PROGRAM_CACHE["nc"] = build_program()
    nc = _PROGRAM_CACHE["nc"]

    in_maps = make_in_maps(x, W_qkv, W_out)
    res = bass_utils.run_bass_kernel_spmd(
        nc, in_maps, core_ids=list(range(NCORES)), trace=_trace, tmpdir=_tmpdir
    )
    out = np.zeros((B, T, D), np.float32)
    for c in range(NCORES):
        b = c // 2
        y = np.asarray(res.results[c]["y"], dtype=np.float32)  # [2, 32, 128, 132]
        for i in range(2):
            yi = y[i].reshape(T, 132)
            out[b] += yi[:, 0:128] / yi[:, 128:129]
    if _trace:
        kernel.last_result = res
    return out



# revision 43
# speedup vs baseline: 1.3076x; 1.0207x over previous
"""Trainium2 Bass kernel: causal multi-head self-attention (B=4, T=4096, D=128, H=4, dh=32).

Sharding: 8 cores = 4 batches x 2 head-pairs. Core c handles batch c//2, heads
{2*(c%2), 2*(c%2)+1}. Each core emits per-head unnormalized projections Y_h and
softmax denominators l_h; the host computes sum_h Y_h / l_h per batch.

S matmuls bf16 untiled full-array; O matmuls fp8e4m3 DoubleRow (one matmul
per PAIR of j-blocks: the two k-tiles hold the two blocks' V columns, halving
O-matmul columns vs bf16). Per (head, q-super of 512 queries), pairs descending:
  mask       : diag blocks get a prefix matmul accumulating a -30000 causal
               bias tile into S's masked/dead columns BEFORE the S matmul, so
               exp emits zeros there directly. (An affine_select on E breaks
               the DoubleRow consumers: gpsimd RMW writes stay invisible to
               PE/ScalarE readers for ~us despite correct semaphores.)
  S^T[j,q]   = ktz_jb(zero-padded K=128) @ qt -> PSUM pair tile [128,1024]
  E (fp8)    : head 0 + every 4th h1 pair on ScalarE (exp -> fp8e4); other h1
               pairs on VectorE via tensor_scalar round(s*a+b) -> int8 whose
               bits read as fp8e4m3 give 2^(s*log2e) (Schraudolph).
  O^T       += DoubleRow([vxA | vxB], [E_A | E_B]), vx block = [V8 | ones |
               Vres8 | 0] with Vres8 = fp8(V - fp8(V)): out rows 0:32 = Y8,
               row 32 = softmax denominator l, rows 33:65 = Y residual.
  proj       : wo rows 0:32 AND 33:65 = W_out^T (sums Y8 + Yres for free),
               ones at (32, 128) extracts l.
qkv runs one unit ahead (depth-2 xt prefetch) so early units never stall on
their own qkv evacuation chains.
"""

import math
import numpy as np

import concourse.bass as bass
import concourse.bacc as bacc
import concourse.mybir as mybir
import concourse.tile as tile
from concourse import bass_utils
import concourse.dve_ops as dve_ops
from concourse.dve_spec import Spec, Src0, C0, C1, relu, lower
from concourse.dve_uop import DveOpSpec

F32 = mybir.dt.float32
BF16 = mybir.dt.bfloat16
FP16 = mybir.dt.float16
FP8 = mybir.dt.float8e4
I16 = mybir.dt.int16
I8 = mybir.dt.int8
DRow = mybir.MatmulPerfMode.DoubleRow
Alu = mybir.AluOpType
Exp = mybir.ActivationFunctionType.Exp

B, T, D = 4, 4096, 128
H, DH = 4, 32
NCORES = 8
NQS = T // 512
SCALE = 1.0 / math.sqrt(DH)

# Schraudolph fp8e4m3-bit exp: int8_bits(e^s) ~= round(s*log2e*8 + (7+sigma)*8)
SIGMA = -0.03
SCHR_A = (1.0 / math.log(2.0)) * SCALE * 8.0
SCHR_B = (7.0 + SIGMA) * 8.0
MASKNEG = -30000.0


def _register_exp2():
    name = "EXP2_BITS_ANT"
    for op in dve_ops.OPS:
        if op.name == name:
            return op
    spec = Spec(body=relu(Src0 * C0 + C1))
    row = dve_ops._CUSTOM_DVE_ROW_BASE + len(dve_ops.OPS)
    assert row < 0x20
    shas = {}
    for ver in ("v3", "v4"):
        try:
            s = DveOpSpec(name=name, opcode=row, uops=lower(spec, ver=ver), rd1_en=False)
            shas[ver] = s.sha(ver)
        except Exception:
            pass
    dve_ops._SUB_OPCODE_FOR_NAME[name] = row
    op = dve_ops.DveOp(name, spec, subdim=False, uops_sha=shas)
    dve_ops.OPS.append(op)
    dve_ops.CUSTOM_DVE_SPECS[name] = spec
    return op


EXP2_OP = _register_exp2()


def build_program() -> bacc.Bacc:
    nc = bacc.Bacc("TRN2", target_bir_lowering=False, debug=False, num_devices=NCORES)

    xt_d = nc.dram_tensor("xt", [D, T], BF16, kind="ExternalInput").ap()
    wqk_d = [nc.dram_tensor(f"wqk{h}", [D, 64], BF16, kind="ExternalInput").ap() for h in range(2)]
    wv_d = nc.dram_tensor("wv", [D, 64], BF16, kind="ExternalInput").ap()
    wo_d = [nc.dram_tensor(f"wo{h}", [128, 132], FP16, kind="ExternalInput").ap() for h in range(2)]
    zpad_d = nc.dram_tensor("zpad", [96, T], BF16, kind="ExternalInput").ap()
    y_d = nc.dram_tensor("y", [2, NQS * 4, 128, 132], FP16, kind="ExternalOutput").ap()

    with tile.TileContext(nc) as tc:
        with (
            tc.tile_pool(name="const", bufs=1) as cpool,
            tc.tile_pool(name="epool", bufs=8) as epool,
            tc.tile_pool(name="ypool", bufs=4) as ypool,
            tc.tile_pool(name="psS", bufs=3, space="PSUM") as psS,
            tc.tile_pool(name="psO", bufs=2, space="PSUM") as psO,
        ):
            # ---- persistent SBUF ----
            xt = cpool.tile([D, T], BF16)
            wqkm = cpool.tile([D, 128], BF16)
            wv = cpool.tile([D, 64], BF16)
            wo = [cpool.tile([128, 132], FP16, name=f"wo{h}", tag=f"wo{h}") for h in range(2)]
            qt = [cpool.tile([128, T], BF16, name=f"qt{h}", tag=f"qt{h}") for h in range(2)]
            ktz = [cpool.tile([128, T], BF16, name=f"ktz{h}", tag=f"ktz{h}") for h in range(2)]
            vx = [cpool.tile([128, 128 * 32], FP8, name=f"vx{h}", tag=f"vx{h}") for h in range(2)]
            idn = cpool.tile([128, 128], BF16)
            rmask = cpool.tile([128, 1024], BF16)
            osb = [cpool.tile([128, T], FP16, name=f"osb{h}", tag=f"osb{h}") for h in range(2)]

            # ---- init loads ----
            nc.sync.dma_start(xt[:, 0:512], xt_d[:, 0:512])
            for h in range(2):
                nc.sync.dma_start(wqkm[:, 64 * h : 64 * h + 64], wqk_d[h][:, :])
            nc.sync.dma_start(xt[:, 512:1024], xt_d[:, 512:1024])
            for h in range(2):
                nc.scalar.dma_start(wo[h][:, :], wo_d[h][:, :])
            nc.scalar.dma_start(wv[:, :], wv_d[:, :])
            zq = [nc.sync, nc.gpsimd, nc.sync, nc.gpsimd]
            for h in range(2):
                # zero the padded contraction rows once; Q/K copies only write
                # rows 0:32. Issued from four different engine queues so the
                # descriptors dispatch in parallel instead of serializing the
                # startup on the Sync queue.
                zq[2 * h].dma_start(qt[h][32:128, :], zpad_d[:, :])
                zq[2 * h + 1].dma_start(ktz[h][32:128, :], zpad_d[:, :])
                # vx pattern: [V8_j | ones | Vres8 | zeros] per 128-col block.
                nc.gpsimd.memset(vx[h][:, 0:512], 0.0)
                for jb in range(4):
                    nc.gpsimd.memset(vx[h][:, 128 * jb + 32 : 128 * jb + 33], 1.0)
                nc.vector.memset(vx[h][:, 512:4096], 0.0)
                for jb in range(4, 32):
                    nc.gpsimd.memset(vx[h][:, 128 * jb + 32 : 128 * jb + 33], 1.0)
            # identity (for the mask prefix-matmul) and the causal bias tile:
            # rmask[p, u] = MASKNEG where u < p + 512 else 0. Block g of a
            # diag super reads rmask[:, 512-128g : 1024-128g] so that column
            # c of the block sees MASKNEG iff c < 128g + p (dead or above
            # the causal frontier). Built once at init (gpsimd writes have
            # ~30us to drain before first use).
            nc.vector.memset(idn[:, :], 1.0)
            nc.gpsimd.affine_select(idn[:, :], idn[:, :], pattern=[[1, 128]],
                                    compare_op=mybir.AluOpType.is_equal, fill=0.0,
                                    base=0, channel_multiplier=-1)
            nc.vector.memset(rmask[:, :], 0.0)
            nc.gpsimd.affine_select(rmask[:, :], rmask[:, :], pattern=[[1, 1024]],
                                    compare_op=mybir.AluOpType.is_ge, fill=MASKNEG,
                                    base=-512, channel_multiplier=-1)

            def copy_h(h, out, in_):
                """PSUM->SBUF evacuations: head 0 on ScalarE, head 1 on VectorE."""
                if h == 0:
                    nc.scalar.copy(out, in_)
                else:
                    nc.vector.tensor_copy(out, in_)

            def emit_qkv(qs):
                qsl = slice(512 * qs, 512 * (qs + 1))
                # both heads' Q/K in ONE matmul: wqkm cols = [q0|k0|q1|k1]
                p = psS.tile([128, 1024], F32, name="p", tag="s")
                nc.tensor.matmul(p[:, 0:512], wqkm[:, :], xt[:, qsl], start=True, stop=True)
                for h in range(2):
                    copy_h(h, qt[h][0:32, qsl], p[64 * h : 64 * h + 32, 0:512])
                    copy_h(h, ktz[h][0:32, qsl], p[64 * h + 32 : 64 * h + 64, 0:512])
                pv = psS.tile([128, 1024], F32, name="pv", tag="s")
                for k in range(4):
                    jsl = slice(512 * qs + 128 * k, 512 * qs + 128 * (k + 1))
                    nc.tensor.matmul(pv[:, 64 * k : 64 * k + 64], xt[:, jsl], wv[:, :], start=True, stop=True)
                for h in range(2):
                    srcv = pv[:, 0:256].rearrange("p (n s) -> p n s", s=64)[:, :, 32 * h : 32 * h + 32]
                    # block jb lands in slot jb^1 (pairwise swap, negative
                    # stride on the inner pair dim): vx pair slices then read
                    # [odd-block | even-block], matching the e k-tile order
                    seg = vx[h][:, 512 * qs : 512 * (qs + 1)]
                    dv8 = seg.rearrange("p (pr two s) -> p pr two s", two=2, s=128)[:, :, ::-1, 0:32]
                    dvr = seg.rearrange("p (pr two s) -> p pr two s", two=2, s=128)[:, :, ::-1, 33:65]
                    s4 = srcv.rearrange("p (pr two) s -> p pr two s", two=2)
                    nc.vector.tensor_copy(dv8, s4)
                    nc.vector.tensor_tensor(dvr, s4, dv8, Alu.subtract)
                # prefetch two supers ahead (qkv runs one unit early)
                if qs + 2 < NQS:
                    nsl = slice(512 * (qs + 2), 512 * (qs + 3))
                    nc.sync.dma_start(xt[:, nsl], xt_d[:, nsl])

            def emit_attn(qs, mid_cb=None):
                njb = 4 * (qs + 1)
                npairs = njb // 2
                o_ps = [psO.tile([128, 512], F32, name=f"o{h}", tag="o") for h in range(2)]
                s_tiles = {}
                e_tiles = {}
                blocks = list(range(njb - 1, -1, -1))  # descending: diag first
                pairs = [blocks[2 * i : 2 * i + 2] for i in range(npairs)]

                def emit_S(h, gi):
                    s = psS.tile([128, 1024], F32, name="s", tag="s")
                    s_tiles[(h, gi)] = s
                    for k, jb in enumerate(pairs[gi]):
                        g = jb - 4 * qs
                        kt = ktz[h][:, 128 * jb : 128 * (jb + 1)]
                        if g >= 0:
                            # causal+dead bias over [0, 128(g+1)) only: exp()
                            # emits zeros there directly (no affine_select on
                            # E -> no gpsimd write-visibility race with the
                            # fp8 DoubleRow O consumers). S accumulates onto
                            # it in the overlap, and covers the rest alone.
                            w = 128 * (g + 1)
                            lo = 128 * g
                            nc.tensor.matmul(
                                s[:, 512 * k : 512 * k + w],
                                idn[:, :],
                                rmask[:, 512 - lo : 512 - lo + w],
                                start=True,
                                stop=True,
                                skip_group_check=True,
                            )
                            nc.tensor.matmul(
                                s[:, 512 * k + lo : 512 * k + w],
                                kt,
                                qt[h][:, 512 * qs + lo : 512 * qs + w],
                                start=False,
                                stop=True,
                                skip_group_check=True,
                            )
                            if w < 512:
                                nc.tensor.matmul(
                                    s[:, 512 * k + w : 512 * (k + 1)],
                                    kt,
                                    qt[h][:, 512 * qs + w : 512 * (qs + 1)],
                                    start=True,
                                    stop=True,
                                )
                        else:
                            nc.tensor.matmul(
                                s[:, 512 * k : 512 * (k + 1)],
                                kt,
                                qt[h][:, 512 * qs : 512 * (qs + 1)],
                                start=True,
                                stop=True,
                            )

                def emit_E(h, gi):
                    s = s_tiles.pop((h, gi))
                    # one fp8 E tile per pair: [E_blk0 | E_blk1]. Masked/dead S
                    # columns hold MASKNEG so exp/schraudolph emit 0 / -0 bits;
                    # no post-mask pass needed (the affine_select's laggy gpsimd
                    # writes were what broke DoubleRow consumers).
                    e = epool.tile([128, 1024], FP8, name="e", tag="e")
                    e_tiles[(h, gi)] = e
                    if h == 0 or gi % 8 == 1:
                        nc.scalar.activation(e[:, :], s[:, :], Exp, scale=SCALE)
                    else:
                        nc.vector.tensor_scalar(e[:, :].bitcast(I8), s[:, :], SCHR_A, SCHR_B, Alu.mult, Alu.add)

                def emit_O(h, gi):
                    e = e_tiles.pop((h, gi))
                    # ONE fp8 DoubleRow matmul per pair covers BOTH j-blocks
                    # (k-tiles = the two blocks' vx columns); out rows
                    # 0:32=Y8, 32=l, 33:65=Y-residual from the fp8 V error.
                    jb0 = pairs[gi][1]  # lower/even block index
                    lhs = vx[h][:, 128 * jb0 : 128 * (jb0 + 2)].rearrange("p (two m) -> p two m", two=2)
                    nc.tensor.matmul(
                        o_ps[h][:, :],
                        lhs,
                        e[:, :].rearrange("p (two n) -> p two n", two=2),
                        start=(gi == 0),
                        stop=(gi == npairs - 1),
                        perf_mode=DRow,
                    )

                emit_S(0, 0)
                emit_S(1, 0)
                for gi in range(npairs):
                    emit_E(0, gi)
                    emit_E(1, gi)
                    if gi + 1 < npairs:
                        emit_S(0, gi + 1)
                        emit_S(1, gi + 1)
                    emit_O(0, gi)
                    # O(h1) deferred three periods: by then its Schraudolph E
                    # is long done, so the in-order PE queue never stalls on
                    # VectorE (the DR O-matmuls are half as long, so the PE
                    # arrives sooner; epool's 8 slots cover the deeper window)
                    if gi > 2:
                        emit_O(1, gi - 3)
                    if mid_cb is not None and gi == npairs // 2:
                        mid_cb()
                if npairs >= 3:
                    emit_O(1, npairs - 3)
                emit_O(1, npairs - 2)
                emit_O(1, npairs - 1)
                return o_ps

            def emit_osb(qs, o_ps):
                qsl = slice(512 * qs, 512 * (qs + 1))
                for h in range(2):
                    # both osb evacuations on VectorE: on ScalarE this copy sits
                    # just ahead of the next unit's first exp and blocks it
                    nc.vector.tensor_copy(osb[h][:, qsl], o_ps[h][:, :])

            def emit_proj(qs):
                for h in range(2):
                    p = psS.tile([128, 1024], F32, name="pp", tag="s")
                    for lqb in range(4):
                        qb = 4 * qs + lqb
                        nc.tensor.matmul(
                            p[:, 256 * lqb : 256 * lqb + 132],
                            osb[h][:, 128 * qb : 128 * (qb + 1)],
                            wo[h][:, :],
                            start=True,
                            stop=True,
                        )
                    yb = ypool.tile([128, 4, 132], FP16, name="yb", tag="y")
                    src = p[:, 0:1024].rearrange("p (n s) -> p n s", s=256)[:, :, 0:132]
                    copy_h(h, yb[:, :, :], src)
                    dst = y_d[h, 4 * qs : 4 * qs + 4].rearrange("n p c -> p n c")
                    nc.sync.dma_start(dst, yb[:, :, :])

            with nc.named_scope("attn"):
                o_prev = None
                for qs in range(NQS):
                    if qs == 0:
                        # depth-2 prefetch: qt/ktz/vx for qs+1 ready when unit
                        # qs starts, so early units never stall on qkv chains
                        emit_qkv(0)
                        emit_qkv(1)
                    if qs > 0:
                        emit_osb(qs - 1, o_prev)
                    # qkv for qs+2 emitted mid-unit so its PSUM->SBUF copies
                    # overlap this unit instead of stalling the boundary
                    cb = (lambda q=qs: emit_qkv(q + 2)) if qs + 2 < NQS else None
                    o_cur = emit_attn(qs, cb)
                    if qs > 0:
                        emit_proj(qs - 1)
                    o_prev = o_cur
                emit_osb(NQS - 1, o_prev)
                emit_proj(NQS - 1)

    nc.compile()
    return nc


def _to_bf16(x: np.ndarray) -> np.ndarray:
    import ml_dtypes

    return np.ascontiguousarray(x, dtype=np.float32).astype(ml_dtypes.bfloat16)


def make_in_maps(x: np.ndarray, W_qkv: np.ndarray, W_out: np.ndarray):
    x = np.asarray(x, dtype=np.float32)
    W_qkv = np.asarray(W_qkv, dtype=np.float32)
    W_out = np.asarray(W_out, dtype=np.float32)

    in_maps = []
    for c in range(NCORES):
        b = c // 2
        h0 = 2 * (c % 2)
        m = {"xt": _to_bf16(x[b].T), "zpad": _to_bf16(np.zeros((96, T), np.float32))}
        for i in range(2):
            h = h0 + i
            wqk = np.zeros((D, 64), np.float32)
            wqk[:, 0:32] = W_qkv[32 * h : 32 * h + 32, :].T
            wqk[:, 32:64] = W_qkv[128 + 32 * h : 128 + 32 * h + 32, :].T
            m[f"wqk{i}"] = _to_bf16(wqk)
            woi = np.zeros((128, 132), np.float32)
            # rows 0:32 multiply Y8 (fp8 V), rows 33:65 the fp8 V-residual
            woi[0:32, 0:128] = W_out[:, 32 * h : 32 * h + 32].T
            woi[33:65, 0:128] = W_out[:, 32 * h : 32 * h + 32].T
            woi[32, 128] = 1.0
            m[f"wo{i}"] = woi.astype(np.float16)
        m["wv"] = _to_bf16(W_qkv[256 + 32 * h0 : 256 + 32 * h0 + 64, :].T)
        in_maps.append(m)
    return in_maps


_PROGRAM_CACHE = {}


def kernel(x: np.ndarray, W_qkv: np.ndarray, W_out: np.ndarray, _trace=False, _tmpdir=None) -> np.ndarray:
    if "nc" not in _PROGRAM_CACHE:
        _PROGRAM_CACHE["nc"] = build_program()
    nc = _PROGRAM_CACHE["nc"]

    in_maps = make_in_maps(x, W_qkv, W_out)
    res = bass_utils.run_bass_kernel_spmd(
        nc, in_maps, core_ids=list(range(NCORES)), trace=_trace, tmpdir=_tmpdir
    )
    out = np.zeros((B, T, D), np.float32)
    for c in range(NCORES):
        b = c // 2
        y = np.asarray(res.results[c]["y"], dtype=np.float32)  # [2, 32, 128, 132]
        for i in range(2):
            yi = y[i].reshape(T, 132)
            out[b] += yi[:, 0:128] / yi[:, 128:129]
    if _trace:
        kernel.last_result = res
    return out

